# revision 6
# baseline (speedup 1.0000x reference)
"""Correspondence contrastive loss on 8 Trainium2 NeuronCores.

Strategy (per sharding hint): replicate the two feature volumes on every
core in HBM, shard the 4096 point-triples 512-per-core. Each core:
  1. loads its 512 point triples (raw integer coords staged as exact f32),
  2. computes p % crop and the flat voxel index on-device (DVE),
  3. indirect-DMA gathers the 32-channel feature rows from the volumes
     (volumes staged in [XYZ, C] layout so each point is one contiguous
     128B row),
  4. computes pos/neg squared distances, sqrt, margin-relu terms,
  5. writes per-point pos_dis/neg_dis and per-partition partial sums.
Host sums the 8 cores' partials (the scalar all-reduce) and assembles the
full outputs.

HW notes discovered while bringing this up:
  - indirect_dma_start consumes ONE offset per partition per instruction
    (extra offset-table columns are silently ignored, and oversized dest
    APs read past the table end -> NRT crash), so each 128-point row
    gather is its own instruction: 12 per core. SWDGE descriptor
    generation (~1us/instruction on the GPSIMD Q7) is the kernel floor.
  - tensor_tensor_reduce (fused InstTensorTensorReduce) crashes the NRT
    runtime -> use plain DVE ops / ACT accum_out instead.
  - Gathers are ordered fixed, positive, negative so the positive-path
    distance math overlaps the negative gathers' descriptor generation.
"""

import numpy as np

import concourse.bass as bass
import concourse.tile as tile
from concourse import bacc, mybir
from concourse import bass_utils

C = 32
X, Y, Z = 100, 88, 80
XYZ = X * Y * Z
N = 4096
NCORES = 8
NPC = N // NCORES  # 512 points per core
P = 128
T = NPC // P       # 4 tiles of 128 points per core
NJ = 3 * T         # 12 (type, tile) pairs; type 0=fixed, 1=positive, 2=negative
SCOLS = 74         # staged free dim: 36 coords + 36 crop + margin + pad
F32 = mybir.dt.float32
I32 = mybir.dt.int32
ALU = mybir.AluOpType
ACT = mybir.ActivationFunctionType

_NC_CACHE = []


def build_nc(reps=1):
    nc = bacc.Bacc("TRN2", target_bir_lowering=False, debug=False,
                   enable_asserts=False)

    staged = nc.dram_tensor("staged", [P, SCOLS], F32, kind="ExternalInput")
    fix2 = nc.dram_tensor("fix2", [XYZ, C], F32, kind="ExternalInput")
    mov2 = nc.dram_tensor("mov2", [XYZ, C], F32, kind="ExternalInput")
    # packed per-core output: cols 0:4 pos_dis, 4:8 neg_dis,
    # 8 sum(pos_d2), 9 sum(relu(margin-neg_dis)^2)
    out_o = nc.dram_tensor("out_o", [P, 10], F32, kind="ExternalOutput")

    with tile.TileContext(nc) as tc:
        with tc.tile_pool(name="pool", bufs=1) as pool:
          for _rep in range(reps):
            st = pool.tile([P, SCOLS], F32)
            nc.sync.dma_start(st[:], staged.ap())

            coords = st[:, 0:36]      # x block | y block | z block, 12 cols each
            crop = st[:, 36:72]       # 100.0 x12 | 88.0 x12 | 80.0 x12
            marg = st[:, 72:73]

            # p % crop for p in [0, 3*crop): two rounds of p -= crop*(p >= crop)
            ge = pool.tile([P, 36], F32)
            tmp = pool.tile([P, 36], F32)
            pm = pool.tile([P, 36], F32)
            nc.vector.tensor_tensor(ge[:], coords, crop, op=ALU.is_ge)
            nc.vector.tensor_mul(tmp[:], ge[:], crop)
            nc.vector.tensor_sub(pm[:], coords, tmp[:])
            nc.vector.tensor_tensor(ge[:], pm[:], crop, op=ALU.is_ge)
            nc.vector.tensor_mul(tmp[:], ge[:], crop)
            nc.vector.tensor_sub(pm[:], pm[:], tmp[:])

            # flat voxel index = x*(Y*Z) + y*Z + z, exact in f32 (< 2^24)
            xm, ym, zm = pm[:, 0:12], pm[:, 12:24], pm[:, 24:36]
            acc = pool.tile([P, NJ], F32)
            idxf = pool.tile([P, NJ], F32)
            nc.vector.scalar_tensor_tensor(acc[:], xm, float(Y * Z), zm,
                                           op0=ALU.mult, op1=ALU.add)
            nc.vector.scalar_tensor_tensor(idxf[:], ym, float(Z), acc[:],
                                           op0=ALU.mult, op1=ALU.add)
            idxi = pool.tile([P, NJ], I32)
            nc.vector.tensor_copy(idxi[:], idxf[:])

            # gathers: ftile[p, t*32:(t+1)*32] = fix2[idx[p, t], :]
            #          mp/mn[p, t*32:(t+1)*32] = mov2[idx[p, T/2T + t], :]
            # Separate pos/neg tiles so the positive-path compute only
            # depends on the first 8 gathers, overlapping the negative
            # gathers' descriptor generation.
            ftile = pool.tile([P, T * C], F32)
            mp = pool.tile([P, T * C], F32)
            mn = pool.tile([P, T * C], F32)
            for t in range(T):
                nc.gpsimd.indirect_dma_start(
                    out=ftile[:, t * C:(t + 1) * C], out_offset=None,
                    in_=fix2.ap(),
                    in_offset=bass.IndirectOffsetOnAxis(
                        ap=idxi[:, t:t + 1], axis=0))
            for t in range(T):  # positive
                nc.gpsimd.indirect_dma_start(
                    out=mp[:, t * C:(t + 1) * C], out_offset=None,
                    in_=mov2.ap(),
                    in_offset=bass.IndirectOffsetOnAxis(
                        ap=idxi[:, T + t:T + t + 1], axis=0))

            OUT = pool.tile([P, 10], F32)
            dis = OUT[:, 0:8]
            S = OUT[:, 8:10]

            # positive path (overlaps negative gathers below)
            dp = pool.tile([P, T * C], F32)
            sp = pool.tile([P, T * C], F32)
            p2 = pool.tile([P, T], F32)
            nc.vector.tensor_sub(dp[:], ftile[:], mp[:])
            nc.vector.tensor_mul(sp[:], dp[:], dp[:])
            nc.vector.tensor_reduce(p2[:],
                                    sp[:].rearrange("p (t c) -> p t c", c=C),
                                    axis=mybir.AxisListType.X, op=ALU.add)
            nc.scalar.sqrt(dis[:, 0:T], p2[:])
            nc.vector.tensor_reduce(S[:, 0:1], p2[:],
                                    axis=mybir.AxisListType.X, op=ALU.add)

            for t in range(T):  # negative
                nc.gpsimd.indirect_dma_start(
                    out=mn[:, t * C:(t + 1) * C],
                    out_offset=None, in_=mov2.ap(),
                    in_offset=bass.IndirectOffsetOnAxis(
                        ap=idxi[:, 2 * T + t:2 * T + t + 1], axis=0))

            dn = pool.tile([P, T * C], F32)
            sn = pool.tile([P, T * C], F32)
            n2 = pool.tile([P, T], F32)
            nc.vector.tensor_sub(dn[:], ftile[:], mn[:])
            nc.vector.tensor_mul(sn[:], dn[:], dn[:])
            nc.vector.tensor_reduce(n2[:],
                                    sn[:].rearrange("p (t c) -> p t c", c=C),
                                    axis=mybir.AxisListType.X, op=ALU.add)
            nc.scalar.sqrt(dis[:, T:2 * T], n2[:])
            # relu(margin - neg_dis), then Square with free-dim accumulation
            nt = pool.tile([P, T], F32)
            nt2 = pool.tile([P, T], F32)
            nc.scalar.activation(nt[:], dis[:, T:2 * T], ACT.Relu,
                                 bias=marg, scale=-1.0)
            nc.scalar.activation(nt2[:], nt[:], ACT.Square,
                                 accum_out=S[:, 1:2])

            nc.sync.dma_start(out_o.ap(), OUT[:])

    nc.compile()
    return nc


def get_nc():
    if not _NC_CACHE:
        _NC_CACHE.append(build_nc())
    return _NC_CACHE[0]


def prep_in_maps(fix_image_feature, moving_image_feature, fixed_points,
                 positive_points, negative_points, margin):
    fix = np.asarray(fix_image_feature, dtype=np.float32).reshape(C, XYZ)
    mov = np.asarray(moving_image_feature, dtype=np.float32).reshape(C, XYZ)
    fix2 = np.ascontiguousarray(fix.T)
    mov2 = np.ascontiguousarray(mov.T)

    staged = np.zeros((NCORES, P, SCOLS), dtype=np.float32)
    for ty, pts in enumerate([fixed_points, positive_points, negative_points]):
        pts = np.asarray(pts).astype(np.float32)  # coords in [0, 200): exact
        seg = pts.reshape(NCORES, T, P, 3)
        for t in range(T):
            j = ty * T + t
            staged[:, :, j] = seg[:, t, :, 0]
            staged[:, :, 12 + j] = seg[:, t, :, 1]
            staged[:, :, 24 + j] = seg[:, t, :, 2]
    staged[:, :, 36:48] = float(X)
    staged[:, :, 48:60] = float(Y)
    staged[:, :, 60:72] = float(Z)
    staged[:, :, 72] = np.float32(margin)

    return [{"staged": staged[k], "fix2": fix2, "mov2": mov2}
            for k in range(NCORES)]


def assemble(results):
    pos_dis = np.concatenate(
        [results[k]["out_o"][:, 0:T].T.reshape(-1) for k in range(NCORES)])
    neg_dis = np.concatenate(
        [results[k]["out_o"][:, T:2 * T].T.reshape(-1) for k in range(NCORES)])
    s_pos = float(sum(results[k]["out_o"][:, 8].astype(np.float64).sum()
                      for k in range(NCORES)))
    s_neg = float(sum(results[k]["out_o"][:, 9].astype(np.float64).sum()
                      for k in range(NCORES)))
    loss = np.float32((s_pos + s_neg) / (2.0 * (2 * N)) * 100.0)
    return loss, pos_dis.astype(np.float32), neg_dis.astype(np.float32)


def kernel(fix_image_feature, moving_image_feature, fixed_points,
           positive_points, negative_points, margin, **run_kwargs):
    nc = get_nc()
    in_maps = prep_in_maps(fix_image_feature, moving_image_feature,
                           fixed_points, positive_points, negative_points,
                           margin)
    res = bass_utils.run_bass_kernel_spmd(
        nc, in_maps, core_ids=list(range(NCORES)), **run_kwargs)
    out = assemble(res.results)
    kernel.last_results = res
    return out
